# revision 99
# baseline (speedup 1.0000x reference)
"""DisorderedCausalSelfAttention on 8 Trainium2 NeuronCores — bf16 pipeline.

Problem: y = proj(causal_attn(rope_bias(qkv(x)))) with
  B=2, T=2048, C=1024, NH=16, D=64, RD=32 (partial RoPE), per-head
  additive biases bQ/bK applied post-RoPE.

Sharding: core c -> (batch b = c//4, head-group g = c%4 of 4 heads).
Each core computes qkv for its 4 heads, attention, and a partial output
projection (its 256 rows of W_proj); the host sums the 4 partials per
batch and adds b_proj.

Design notes (TimelineSim: 145.2us/core vs 195.9us for the f32r
baseline; hardware rel_err 4.9e-3 vs the 2e-2 gate):
  - Everything stored bf16 (fp8 measured to fail: 2.1e-2 for fp8 QK^T,
    0.27 for fp8 AV - softmax weights underflow e4m3); PSUM accumulation
    stays f32.  bf16 halves DMA, doubles DVE element-wise throughput
    (2x_1p), and removes the f32r <256-wide matmul penalty so the QK^T
    matmuls trim to the causal boundary like exp/AV.
  - ACT engine runs ONLY Exp as activation (one table load ever, pre-
    warmed during the DMA head); its other work uses TensorScalar(+0)
    which touches no tables.  GPSIMD cannot access PSUM (BIR verifier),
    so all PSUM drains sit on DVE/ACT; Pool keeps the SBUF memset and
    SWDGE dispatches.
  - One PSUM pool for the whole kernel: tag "pa" ([128,512], ring of 2)
    serves QKV accumulation, rope PERM products, V projection (two
    k-tiles share a tile), and the output projection; tag "s"
    ([128,2,512], ring of 2) the attention logits (both heads side by
    side -> one wide exp per k-tile); tag "y" ([128,512] per head, ring
    of 2) the AV accumulators.  8 banks exactly, so every phase can
    overlap every other.
  - Emission is fully interleaved: block t = {q01/k01 proj+rope for t,
    q23/k23 for t-1, attention q-tile (0,t), attention q-tile (1,t-1)};
    V k-tiles project inline right before their first AV use; the output
    projection drains as PE filler behind each (1,qt)'s normalize.  Both
    exp streams (73us total, ACT-bound) spread under the PE timeline
    (118us busy); a 20-deep p-ring keeps exps flowing when one stream's
    AVs trail a q-tile behind.
  - x^T loads t-major on one queue in strict consumption order (the
    cost model's DMA FIFO is dispatch-ordered), so the first QKV tile
    completes after ~1/4 of the x bytes.

The whole kernel needs exactly zero on-device transposes.
"""

import sys

sys.path.insert(0, "/opt/trn_rl_repo")

import json

import ml_dtypes
import numpy as np

B, T, C, NH, D, RD = 2, 2048, 1024, 16, 64, 32
G = 4  # head-groups (cores per batch)
HPG = NH // G  # heads per group = 4
N_CORES = 8
SCALE = float(D) ** -0.5

_cache = {}


# ---------------------------------------------------------------------------
# Workaround: this container's walrus build accepts at most ONE sync-wait
# command on most instructions, while Tile emits up to ~4.  Split excess
# waits into EventSemaphore instructions inserted immediately before, on the
# same engine (same-queue program order keeps semantics).
# ---------------------------------------------------------------------------
def _split_waits(bj: bytes, es_cap: int = 2) -> bytes:
    d = json.loads(bj)
    for fn in d.get("functions", []):
        for bb in fn.get("blocks", []):
            new = []
            for inst in bb.get("instructions", []):
                si = inst.get("sync_info") or {}
                w = si.get("on_wait") or []
                lim = es_cap if inst.get("opcode") == "EventSemaphore" else 1
                if len(w) > lim:
                    keep = w[-lim:]
                    mv = w[:-lim]
                    for ci in range(0, len(mv), es_cap):
                        new.append({
                            "debug": inst.get("debug"),
                            "engine": inst["engine"],
                            "ins": [], "outs": [],
                            "name": f"{inst['name']}_ws{ci}",
                            "opcode": "EventSemaphore",
                            "sync_info": {"on_update": [],
                                          "on_wait": mv[ci:ci + es_cap]},
                        })
                    si["on_wait"] = keep
                new.append(inst)
            bb["instructions"] = new
    return json.dumps(d).encode()


def _install_waitsplit():
    from concourse import bass2jax, bass_utils

    if getattr(bass2jax.compile_bir_kernel, "_waitsplit", False):
        return
    orig = bass_utils.compile_bir_kernel

    def patched(bj, tmpdir, neff_name="file.neff"):
        return orig(_split_waits(bj), tmpdir, neff_name)

    patched._waitsplit = True
    bass2jax.compile_bir_kernel = patched


# ---------------------------------------------------------------------------
# Kernel builder (one SPMD program; per-core data differs via in_maps)
# ---------------------------------------------------------------------------
def _build(loop_k: int = 1):
    import concourse.bass as bass
    import concourse.tile as tile
    from concourse import mybir
    from concourse.alu_op_type import AluOpType

    f32 = mybir.dt.float32
    bf16 = mybir.dt.bfloat16
    Exp = mybir.ActivationFunctionType.Exp

    nc = bass.Bass("TRN2")

    xT = nc.declare_dram_parameter("x_T", [C, T], bf16, isOutput=False)
    wqk = nc.declare_dram_parameter("w_qk", [C, 2 * HPG * D], bf16, isOutput=False)
    wv = nc.declare_dram_parameter("w_v", [C, HPG * D], bf16, isOutput=False)
    wp = nc.declare_dram_parameter("w_p", [HPG * D, C], bf16, isOutput=False)
    cosr = nc.declare_dram_parameter("cos_r", [128, T], bf16, isOutput=False)
    sinr = nc.declare_dram_parameter("sin_r", [128, T], bf16, isOutput=False)
    bqk = nc.declare_dram_parameter("bias_qk", [128, 4], f32, isOutput=False)
    trim = nc.declare_dram_parameter("tri", [128, 128], bf16, isOutput=False)
    perm = nc.declare_dram_parameter("perm", [128, 128], bf16, isOutput=False)
    out = nc.declare_dram_parameter("out", [T, C], bf16, isOutput=True)

    NT = T // 512       # 4 q/t tiles of 512
    NK = T // 128       # 16 k tiles of 128
    NC_ = C // 128      # 8 contract chunks

    wqk_r = wqk.rearrange("(c p) n -> p c n", p=128)
    wv_r = wv.rearrange("(c p) n -> p c n", p=128)
    wp_r = wp.rearrange("(c p) n -> p c n", p=128)

    with tile.TileContext(nc) as tc:
      for _rep in range(loop_k):
        with tc.tile_pool(name="persist", bufs=1) as pp:
            WQK = pp.tile([128, NC_, 512], bf16)
            WV = pp.tile([128, NC_, 256], bf16)
            WP = pp.tile([128, 2, 1024], bf16)
            BQK = pp.tile([128, 4], f32)
            TRI = pp.tile([128, 128], bf16)
            QK = pp.tile([128, 4, T], bf16)         # chunks: q01,q23,k01,k23
            V4 = pp.tile([128, NK, HPG, 2 * D], bf16)
            YT = pp.tile([128, 2, T], bf16)

            nc.gpsimd.memset(V4[:, :, :, D:], 1.0)  # SBUF-only: Pool is legal

            with tc.tile_pool(name="xtp", bufs=1) as xp:
                XT = xp.tile([128, NC_, T], bf16)
                COS = xp.tile([128, T], bf16)
                SIN = xp.tile([128, T], bf16)
                PERM = xp.tile([128, 128], bf16)
                TMP = xp.tile([128, T], bf16)
                # loads in consumption order.  WQK first (gates the first
                # matmul), then x^T t-major so QKV tile t completes after
                # 1/4 of the x bytes; tables interleaved at first use.
                # One FIFO, strict consumption order (multi-queue dispatch
                # races reorder the global DMA FIFO, so phase-A-critical
                # loads all go on the sync queue back-to-back).
                xT_r = xT.rearrange("(c p) n -> p c n", p=128)
                # phase A touches only the q01 (cols 0:128) and k01
                # (256:384) weight blocks: land those plus the first x
                # t-block first; the q23/k23 blocks ride behind the last
                # x block (needed ~23us in, land ~20us)
                nc.sync.dma_start(out=WQK[:, :, 0:128], in_=wqk_r[:, :, 0:128])
                nc.sync.dma_start(out=XT[:, 0:4, 0:512], in_=xT_r[:, 0:4, 0:512])
                nc.sync.dma_start(out=XT[:, 4:8, 0:512], in_=xT_r[:, 4:8, 0:512])
                nc.sync.dma_start(out=WQK[:, :, 256:384],
                                  in_=wqk_r[:, :, 256:384])
                nc.sync.dma_start(out=PERM, in_=perm[:, :])
                nc.sync.dma_start(out=COS, in_=cosr[:, :])
                nc.sync.dma_start(out=SIN, in_=sinr[:, :])
                nc.sync.dma_start(out=BQK, in_=bqk[:, :])
                nc.sync.dma_start(out=WV, in_=wv_r)
                for t in range(1, NT):
                    nc.sync.dma_start(
                        out=XT[:, :, t * 512:(t + 1) * 512],
                        in_=xT_r[:, :, t * 512:(t + 1) * 512])
                nc.sync.dma_start(out=WQK[:, :, 128:256],
                                  in_=wqk_r[:, :, 128:256])
                nc.sync.dma_start(out=WQK[:, :, 384:512],
                                  in_=wqk_r[:, :, 384:512])
                nc.gpsimd.dma_start(out=TRI, in_=trim[:, :])
                nc.gpsimd.dma_start(out=WP, in_=wp_r)

                with (
                    tc.tile_pool(name="ps", bufs=2, space="PSUM") as ps,
                    tc.tile_pool(name="att", bufs=3) as ap,
                    tc.tile_pool(name="op", bufs=3) as op,
                ):
                    # ---- emission helpers (all share the 8-bank PSUM pool) --
                    def proj_rope(m, t):
                        """QKV projection + RoPE + bias for one (chunk, t-tile)."""
                        tc_ = slice(t * 512, (t + 1) * 512)
                        pa = ps.tile([128, 512], f32, tag="pa", name=f"pa_{m}_{t}")
                        for c in range(NC_):
                            nc.tensor.matmul(
                                pa,
                                WQK[:, c, m * 128:(m + 1) * 128],
                                XT[:, c, tc_],
                                start=(c == 0), stop=(c == NC_ - 1),
                            )
                        if m == 1:
                            nc.vector.tensor_copy(QK[:, m, tc_], pa)
                        else:
                            nc.scalar.add(QK[:, m, tc_], pa, 0.0)
                        # RoPE: swapped rot halves via PE matmul with a
                        # host-built permutation matrix (zero rows on pass
                        # dims); SIN host-signed with zero pass rows, COS has
                        # ones on pass rows -> whole-partition vector ops.
                        pr = ps.tile([128, 512], f32, tag="pa", name=f"pr_{m}_{t}")
                        nc.tensor.matmul(pr, PERM, QK[:, m, tc_],
                                         start=True, stop=True)
                        with tc.high_priority(offset=6):
                            nc.vector.tensor_mul(TMP[:, tc_], pr, SIN[:, tc_])
                            nc.vector.tensor_mul(QK[:, m, tc_], QK[:, m, tc_],
                                                 COS[:, tc_])
                            nc.vector.tensor_add(QK[:, m, tc_],
                                                 QK[:, m, tc_], TMP[:, tc_])
                            nc.vector.tensor_scalar_add(
                                QK[:, m, tc_], QK[:, m, tc_], BQK[:, m:m + 1])

                    vshare = {}

                    def vproj(t):
                        """V projection for one 128-row k-tile.  Pairs of
                        k-tiles share one pa-ring tile (halves ring churn);
                        the PSUM drains alternate DVE / ACT."""
                        if t % 2 == 0:
                            vshare[0] = ps.tile([128, 512], f32, tag="pa",
                                                name=f"pv_{t}")
                        pv = vshare[0][:, 256 * (t % 2):256 * (t % 2) + 256]
                        for c in range(NC_):
                            nc.tensor.matmul(
                                pv,
                                XT[:, c, t * 128:(t + 1) * 128],
                                WV[:, c, :],
                                start=(c == 0), stop=(c == NC_ - 1),
                            )
                        src = pv.rearrange("p (h d) -> p h d", h=HPG)
                        if t % 2 == 0:
                            nc.vector.tensor_copy(V4[:, t, :, 0:D], src)
                        else:
                            nc.scalar.add(V4[:, t, :, 0:D], src, 0.0)

                    def outproj(t, late=False):
                        """Output projection partial for one 128-row tile.

                        late=True (after the last exp) borrows the then-dead
                        attention s-ring banks and flushes per-half DMAs so
                        the tail drains as fast as possible."""
                        ob = op.tile([128, 1024], bf16, tag="ob", name=f"ob_{t}")
                        po2 = ps.tile([128, 2, 512], f32, tag="s",
                                      name=f"po_{t}") if late else None
                        for n in range(2):
                            po = (po2[:, n, :] if late else
                                  ps.tile([128, 512], f32, tag="pa",
                                          name=f"po_{t}_{n}"))
                            for c in range(2):
                                nc.tensor.matmul(
                                    po,
                                    YT[:, c, t * 128:(t + 1) * 128],
                                    WP[:, c, n * 512:(n + 1) * 512],
                                    start=(c == 0), stop=(c == 1),
                                )
                            # PSUM drains alternate DVE / ACT (TensorScalar
                            # +0, no table traffic) to keep the pa ring
                            # moving without serializing either engine
                            if n == 1:
                                nc.scalar.add(ob[:, n * 512:(n + 1) * 512],
                                              po, 0.0)
                            else:
                                nc.vector.tensor_copy(
                                    ob[:, n * 512:(n + 1) * 512], po)
                            nc.sync.dma_start(
                                out=out[t * 128:(t + 1) * 128,
                                        n * 512:(n + 1) * 512],
                                in_=ob[:, n * 512:(n + 1) * 512])

                    filler = []  # deferred PE work fed between k-tiles

                    def drain_filler(n):
                        for _ in range(n):
                            if filler:
                                filler.pop(0)()

                    def att_qt(hp, qt, defer_norm=False):
                        """One attention q-tile (both heads of the pair).

                        defer_norm: return the normalize as a closure instead
                        of emitting it (lets the caller slot it after the next
                        block's projection copies, avoiding a DVE priority
                        inversion at block boundaries)."""
                        qc, kc = hp, 2 + hp
                        ys = [ps.tile([128, 512], f32, tag="y",
                                      name=f"y{hi}_{hp}_{qt}")
                              for hi in range(2)]
                        nkt = 4 * qt + 4
                        for kt in range(nkt):
                            j = kt - 4 * qt
                            c0 = max(j, 0) * 128
                            # both heads' S tiles in one 2-bank PSUM group ->
                            # a single wide exp instruction per kt
                            s = ps.tile([128, 2, 512], f32, tag="s",
                                        name=f"s_{hp}_{qt}_{kt}")
                            with tc.high_priority(offset=6):
                                for hi in range(2):
                                    o = hi * 64
                                    nc.tensor.matmul(
                                        s[:, hi, c0:],
                                        QK[o:o + 64, kc,
                                           kt * 128:(kt + 1) * 128],
                                        QK[o:o + 64, qc,
                                           qt * 512 + c0:(qt + 1) * 512],
                                        start=True, stop=True,
                                    )
                            if hp == 0 and j >= 0:
                                # first use of this V k-tile is the AV below:
                                # project it now (keeps PE fed under the exp)
                                vproj(kt)
                            p = ap.tile([128, 2, 512], bf16, tag="p", bufs=20,
                                        name=f"p_{hp}_{qt}_{kt}")
                            nc.scalar.activation(p[:, :, c0:], s[:, :, c0:],
                                                 Exp, scale=SCALE)
                            if j >= 0:
                                # zero strictly-below-diagonal entries of the
                                # boundary block for both heads at once
                                nc.vector.tensor_mul(
                                    p[:, :, c0:c0 + 128], p[:, :, c0:c0 + 128],
                                    TRI[:, None, :].broadcast_to([128, 2, 128]))
                            for hi in range(2):
                                nc.tensor.matmul(
                                    ys[hi][:, c0:],
                                    V4[:, kt, 2 * hp + hi, :],
                                    p[:, hi, c0:],
                                    start=(kt == 0), stop=(kt == nkt - 1),
                                )
                            if kt % 2 == 0:
                                drain_filler(1)
                        # normalize: rows 64:128 of ys hold the softmax
                        # denominators (ones-block matmul), partition-
                        # replicated; divide rows 0:64 by them.
                        def normalize():
                            for hi in range(2):
                                rb = ap.tile([128, 512], f32, tag="rb", bufs=4,
                                             name=f"rb{hi}_{hp}_{qt}")
                                o = hi * 64
                                nc.vector.reciprocal(rb[o:o + 64, :],
                                                     ys[hi][64:128, :])
                                nc.vector.tensor_mul(
                                    YT[o:o + 64, hp, qt * 512:(qt + 1) * 512],
                                    ys[hi][0:D, :], rb[o:o + 64, :])
                        if defer_norm:
                            return normalize
                        normalize()

                    # ---- emission schedule ------------------------------
                    # Warm the ACT exp table during the DMA head so the
                    # first real exp doesn't pay the 1.3us table load.
                    warm = ap.tile([128, 8], f32, tag="warm", name="warm")
                    nc.vector.memset(warm, 0.0)
                    nc.scalar.activation(warm, warm, Exp, scale=1.0)
                    # keep PE continuously busy through the DMA head so the
                    # first real matmuls start at ramped p-state
                    wz = ap.tile([128, 512], bf16, tag="warm2", name="wz")
                    nc.vector.memset(wz, 0.0)
                    pd = ps.tile([128, 2, 512], f32, tag="s", name="pdummy")
                    for _w in range(6):
                        nc.tensor.matmul(pd[:, 0, :], wz[:, 0:128], wz,
                                         start=True, stop=True)

                    # Fully interleaved: block t runs the q01/k01 projections
                    # for t, the q23/k23 projections for t-1, then attention
                    # q-tile (0,t) and (1,t-1) back to back.  Both hp exp
                    # streams spread across the whole PE timeline (ACT 73us
                    # under PE 118us -> exp never binds), V k-tiles project
                    # inline right before first AV use, and the output
                    # projection drains as PE filler behind each (1,qt)'s
                    # normalize.  The deep p-ring keeps exps flowing even
                    # when one stream's AVs trail a q-tile behind.
                    for t in range(NT):
                        proj_rope(0, t)
                        proj_rope(2, t)
                        if t >= 1:
                            proj_rope(1, t - 1)
                            proj_rope(3, t - 1)
                        att_qt(0, t)
                        if t >= 1:
                            att_qt(1, t - 1)
                            filler += [lambda u=u: outproj(u)
                                       for u in range(4 * (t - 1), 4 * t)]
                    proj_rope(1, NT - 1)
                    proj_rope(3, NT - 1)
                    att_qt(1, NT - 1)
                    filler += [lambda u=u: outproj(u)
                               for u in range(4 * (NT - 1), 4 * NT)]
                    drain_filler(len(filler))

    return nc


def _prep_inputs(x, rope_cos, rope_sin, W_attn, b_attn, W_proj, b_proj, bQ, bK):
    """Slice/transpose the full inputs into 8 per-core input maps."""
    assert not np.any(b_attn), "kernel assumes b_attn == 0 (true for this problem)"
    f = np.float32
    bf = ml_dtypes.bfloat16
    in_maps = []
    # per-batch tensors
    xTb = [np.ascontiguousarray(np.asarray(x[b]).T).astype(bf) for b in range(B)]
    cos_r, sin_r = [], []
    for b in range(B):
        ct = np.zeros((128, T), dtype=f)
        st = np.zeros((128, T), dtype=f)
        sT = np.asarray(rope_sin[b]).T  # [RD, T]
        signed = np.concatenate([-sT[0:RD // 2], sT[RD // 2:RD]], axis=0)
        ct[0:RD, :] = np.asarray(rope_cos[b]).T
        ct[64:64 + RD, :] = np.asarray(rope_cos[b]).T
        ct[RD:64, :] = 1.0
        ct[64 + RD:128, :] = 1.0
        st[0:RD, :] = signed
        st[64:64 + RD, :] = signed
        cos_r.append(ct.astype(bf))
        sin_r.append(st.astype(bf))
    tri = np.triu(np.ones((128, 128), dtype=f)).astype(bf)
    pm = np.zeros((128, 128), dtype=f)
    H = RD // 2
    for base in (0, 64):
        for i in range(H):
            pm[base + H + i, base + i] = 1.0      # out[0:16] = in[16:32]
            pm[base + i, base + H + i] = 1.0      # out[16:32] = in[0:16]
    pm = pm.astype(bf)
    W_attn = np.asarray(W_attn)
    W_proj = np.asarray(W_proj)
    bQ = np.asarray(bQ)
    bK = np.asarray(bK)
    for core in range(N_CORES):
        b, g = divmod(core, G)
        qcols = slice(g * HPG * D, (g + 1) * HPG * D)
        w_qk = np.ascontiguousarray(
            np.concatenate(
                [W_attn[:, qcols], W_attn[:, C + g * HPG * D: C + (g + 1) * HPG * D]],
                axis=1), dtype=f).astype(bf)
        w_v = np.ascontiguousarray(
            W_attn[:, 2 * C + g * HPG * D: 2 * C + (g + 1) * HPG * D],
            dtype=f).astype(bf)
        w_p = np.ascontiguousarray(
            W_proj[g * HPG * D:(g + 1) * HPG * D, :], dtype=f).astype(bf)
        bias = np.zeros((128, 4), dtype=f)
        for j in range(4):
            src = bQ if j < 2 else bK
            h0 = g * HPG + (j % 2) * 2
            bias[0:64, j] = src[h0]
            bias[64:128, j] = src[h0 + 1]
        in_maps.append({
            "x_T": xTb[b],
            "w_qk": w_qk,
            "w_v": w_v,
            "w_p": w_p,
            "cos_r": cos_r[b],
            "sin_r": sin_r[b],
            "bias_qk": bias,
            "tri": tri,
            "perm": pm,
        })
    return in_maps


def _get_nc(loop_k: int = 1):
    key = ("nc", loop_k)
    if key not in _cache:
        _install_waitsplit()
        _cache[key] = _build(loop_k)
    return _cache[key]


def run_spmd(in_maps, loop_k: int = 1):
    from concourse.bass_utils import run_bass_kernel_spmd

    nc = _get_nc(loop_k)
    return run_bass_kernel_spmd(nc, in_maps, core_ids=list(range(N_CORES)))


def kernel(x, rope_cos, rope_sin, W_attn, b_attn, W_proj, b_proj, bQ, bK):
    in_maps = _prep_inputs(x, rope_cos, rope_sin, W_attn, b_attn, W_proj, b_proj,
                           bQ, bK)
    res = run_spmd(in_maps)
    outs = [res.results[c]["out"] for c in range(N_CORES)]
    b_proj = np.asarray(b_proj, dtype=np.float64)
    full = np.empty((B, T, C), dtype=np.float32)
    for b in range(B):
        acc = np.zeros((T, C), dtype=np.float64)
        for g in range(G):
            acc += outs[b * G + g].astype(np.float64)
        full[b] = (acc + b_proj).astype(np.float32)
    return full
